# revision 30
# baseline (speedup 1.0000x reference)
"""Trainium2 Bass kernel for nn_CA_Module (DANet CAM + SE gate).

Reference math (per batch item b):
    q = x[b].reshape(C, N)                         # C=512, N=4096
    energy = q @ q.T                               # [C, C]
    att = softmax(max_row(energy) - energy)        # == softmax(-energy) rows
    out = att @ q                                  # [C, N]
    pooled = concat([mean_n x, mean_n out])        # [2C]
    hidden = relu(pooled @ w1.T + b1)              # [CR]
    se = sigmoid(hidden @ w2.T + b2)               # [C]
    y = se * x + (1 - se) * out

Sharding: data-parallel over B=16 across 8 cores (2 batch items/core).

Device implementation highlights:
  - One f32r copy of q serves everything: DMA lands raw f32 bits in the q_r
    tile, an in-place Copy rounds to f32r (satisfying the fp32r-producer
    rule) and its accum_out gives the pooled-x sums for free.
  - energy is computed upper-block-triangle only (it is symmetric); the
    missing blocks are mirrored with PE transposes into the same PSUM banks.
  - att row c: exp(min_row(energy)[c] - energy[c,:]) / S[c]; S comes free
    from the exp's accum_out.
  - final blend folds into the second matmul: y = A @ q with
    A = ((1-se)/S) * E~ + diag(se), so phase D is pure matmul + copy-out.
  - mean_n out is not reduced from the big out tensor: sum_n out_raw =
    E~ @ (sum_n q), a [512,512]@[512,2] matmul against the pooled-x sums.
  - sigmoid is exp(-z) -> +1 -> reciprocal so the ACT engine stays on the
    exp table set the whole kernel (table switches cost ~2.7us).
  - w1t is pre-scaled by 1/N on the host so pooled sums need no extra scale.
"""
import threading
import numpy as np

import concourse.bass as bass
import concourse.tile as tile
from concourse import bacc, mybir, masks
from concourse.bass_utils import run_bass_kernel_spmd

B, C, H, W = 16, 512, 64, 64
N = H * W                 # 4096
NCORES = 8
BP = B // NCORES          # batch items per core
CR = C // 8               # 64
P = 128                   # partitions
CB = C // P               # 4 c-blocks
NK = N // P               # 32 n-blocks of 128
NB = N // 512             # 8 n-chunks of 512
QCH = 8                   # q DMA/cast chunks per c-block
QCW = N // QCH            # chunk width (512)

f32 = mybir.dt.float32
f32r = mybir.dt.float32r
FT = mybir.ActivationFunctionType
ALU = mybir.AluOpType
AX = mybir.AxisListType

_lock = threading.Lock()
_cached = {}


def _build():
    nc = bacc.Bacc("TRN2", target_bir_lowering=False, debug=False,
                   num_devices=NCORES)

    x_d = nc.dram_tensor("x", [BP, C, N], f32, kind="ExternalInput").ap()
    w1t_d = nc.dram_tensor("w1t", [2 * C, CR], f32, kind="ExternalInput").ap()
    b1_d = nc.dram_tensor("b1", [CR, 1], f32, kind="ExternalInput").ap()
    w2t_d = nc.dram_tensor("w2t", [CR, C], f32, kind="ExternalInput").ap()
    b2n_d = nc.dram_tensor("b2n", [C, 1], f32, kind="ExternalInput").ap()
    ident_d = nc.dram_tensor("ident", [P, P], f32, kind="ExternalInput").ap()
    y_d = nc.dram_tensor("y", [BP, C, N], f32, kind="ExternalOutput").ap()

    with tile.TileContext(nc) as tc:
        _emit(nc, tc, x_d, w1t_d, b1_d, w2t_d, b2n_d, ident_d, y_d)
    nc.compile()
    return nc


def _emit(nc, tc, x_d, w1t_d, b1_d, w2t_d, b2n_d, ident_d, y_d):
    from contextlib import ExitStack
    ctx = ExitStack()
    with ctx:
        consts = ctx.enter_context(tc.tile_pool(name="consts", bufs=1))
        pq = ctx.enter_context(tc.tile_pool(name="pq", bufs=2 * CB))
        pst = ctx.enter_context(tc.tile_pool(name="pst", bufs=4))
        pqT = ctx.enter_context(tc.tile_pool(name="pqT", bufs=3))
        pE = ctx.enter_context(tc.tile_pool(name="pE", bufs=4))
        pET = ctx.enter_context(tc.tile_pool(name="pET", bufs=4))
        pA = ctx.enter_context(tc.tile_pool(name="pA", bufs=4))
        pEg = ctx.enter_context(tc.tile_pool(name="pEg", bufs=4))
        pmir = ctx.enter_context(tc.tile_pool(name="pmir", bufs=1))
        pbl = ctx.enter_context(tc.tile_pool(name="pbl", bufs=3))
        psm = ctx.enter_context(tc.tile_pool(name="psm", bufs=8))
        # PSUM: eps(4) + tps(2) + ops(2) = 8 banks
        peps = ctx.enter_context(
            tc.tile_pool(name="peps", bufs=4, space=bass.MemorySpace.PSUM))
        ptps = ctx.enter_context(
            tc.tile_pool(name="ptps", bufs=2, space=bass.MemorySpace.PSUM))
        pops = ctx.enter_context(
            tc.tile_pool(name="pops", bufs=2, space=bass.MemorySpace.PSUM))

        # ---- constants (weights DMA'd later, after the first q chunks) ----
        ident = consts.tile([P, P], f32, tag="ident")
        nc.sync.dma_start(ident[:], ident_d[:])
        identr = consts.tile([P, P], f32r, tag="identr")
        nc.vector.tensor_copy(identr[:], ident[:])

        def emit_weight_loads():
            w1t_sb = consts.tile([P, 2 * C // P, CR], f32, tag="w1t",
                                 name="w1t_sb")
            nc.sync.dma_start(w1t_sb[:],
                              w1t_d.rearrange("(kb p) j -> p kb j", p=P))
            w1tr = consts.tile([P, 2 * C // P, CR], f32r, tag="w1tr",
                               name="w1tr")
            nc.vector.tensor_copy(w1tr[:], w1t_sb[:])

            w2t_sb = consts.tile([CR, C], f32, tag="w2t", name="w2t_sb")
            nc.sync.dma_start(w2t_sb[:], w2t_d[:])
            w2tr = consts.tile([CR, C], f32r, tag="w2tr", name="w2tr")
            nc.vector.tensor_copy(w2tr[:], w2t_sb[:])

            b1_sb = consts.tile([CR, 1], f32, tag="b1", name="b1_sb")
            nc.sync.dma_start(b1_sb[:], b1_d[:])
            b2n_sb = consts.tile([P, CB], f32, tag="b2n", name="b2n_sb")
            nc.sync.dma_start(b2n_sb[:],
                              b2n_d.rearrange("(cb p) one -> p (cb one)", p=P))
            return w1tr, w2tr, b1_sb, b2n_sb

        # ---- per-batch state ----
        qr = {}      # b -> [CB] tiles [P, N] f32r
        pxacc = {}   # b -> [CB] accumulator tiles [P, QCH] f32

        def make_load_pairs(b):
            """Closures, one per (chunk, cb): DMA x chunk -> staging, then
            cast staging -> q_r chunk (accumulating pooled-x row sums)."""
            tiles = []
            for cb in range(CB):
                t = pq.tile([P, N], f32r, tag="q", name=f"q_{b}_{cb}")
                tiles.append(t)
            qr[b] = tiles
            pxacc[b] = [psm.tile([P, QCH], f32, tag="pxacc",
                                 name=f"pxacc_{b}_{cb}") for cb in range(CB)]

            def pair(cb, ch):
                def go(i):
                    st = pst.tile([P, QCW], f32, tag="qst",
                                  name=f"qst_{b}_{cb}_{ch}")
                    nc.sync.dma_start(
                        st[:],
                        x_d[b, cb * P:(cb + 1) * P, ch * QCW:(ch + 1) * QCW])
                    dst = tiles[cb][:, ch * QCW:(ch + 1) * QCW]
                    acc = pxacc[b][cb][:, ch:ch + 1]
                    with nc.allow_low_precision(reason="f32r round of q"):
                        if i % 2 == 0:
                            nc.scalar.activation(dst, st[:], FT.Copy,
                                                 accum_out=acc)
                        else:
                            nc.vector.tensor_scalar(
                                out=dst, in0=st[:], scalar1=1.0, scalar2=0.0,
                                op0=ALU.mult, op1=ALU.add, accum_out=acc)
                return go

            return [pair(cb, ch) for ch in range(QCH) for cb in range(CB)]

        def emit_px(b):
            px = []
            for cb in range(CB):
                pxt = psm.tile([P, 2], f32r, tag="px", name=f"px_{b}_{cb}")
                with nc.allow_low_precision(reason="pooled sums feed SE gate"):
                    nc.vector.tensor_reduce(pxt[:, 0:1], pxacc[b][cb][:],
                                            axis=AX.X, op=ALU.add)
                    nc.vector.tensor_copy(pxt[:, 1:2], pxt[:, 0:1])
                px.append(pxt)
            return px

        E_ps_of = {}

        def phaseB_gen(b, sprinkle=None):
            """Generator: yields after each k iteration (32 yields)."""
            q = qr[b]
            E_ps = [peps.tile([P, C], f32, tag="eps", name=f"E_ps_{b}_{i}")
                    for i in range(CB)]
            E_ps_of[b] = E_ps

            def emit_transpose_block(k):
                t_ps = ptps.tile([P, C], f32r, tag="tps", name=f"t_ps_{b}_{k}")
                for cb in range(CB):
                    nc.tensor.transpose(
                        t_ps[:, cb * P:(cb + 1) * P],
                        q[cb][:, k * P:(k + 1) * P], identr[:])
                qTt = pqT.tile([P, C], f32r, tag="qT", name=f"qT_{b}_{k}")
                if k % 2 == 0:
                    nc.scalar.activation(qTt[:], t_ps[:], FT.Copy)
                else:
                    nc.vector.tensor_copy(qTt[:], t_ps[:])
                return qTt

            prev_qT = emit_transpose_block(0)
            for k in range(NK):
                nxt_qT = emit_transpose_block(k + 1) if k + 1 < NK else None
                for mc in range(CB):
                    nc.tensor.matmul(
                        E_ps[mc][:, mc * P:C],
                        prev_qT[:, mc * P:(mc + 1) * P],
                        prev_qT[:, mc * P:C],
                        start=(k == 0), stop=(k == NK - 1))
                if sprinkle is not None and k < len(sprinkle):
                    sprinkle[k](k)
                prev_qT = nxt_qT
                yield

        def phaseC_gen(b, px, weights, out):
            w1tr, w2tr, b1_sb, b2n_sb = weights
            E_ps = E_ps_of[b]
            # mirror lower-triangle blocks: E[md, mc] = E[mc, md]^T
            for mc in range(CB):
                for md in range(mc + 1, CB):
                    mt = pmir.tile([P, P], f32, tag="mir",
                                   name=f"mir_{b}_{mc}_{md}")
                    nc.vector.tensor_copy(mt[:], E_ps[mc][:, md * P:(md + 1) * P])
                    nc.tensor.matmul(
                        E_ps[md][:, mc * P:(mc + 1) * P], mt[:], ident[:],
                        is_transpose=True, start=False, stop=True)
                yield
            # ---- phase C: softmax pieces ----
            E_sb, rS = [], []
            for mc in range(CB):
                m_sb = psm.tile([P, 1], f32, tag="m", name=f"m_{b}_{mc}")
                nc.vector.tensor_reduce(m_sb[:], E_ps[mc][:], axis=AX.X,
                                        op=ALU.min)
                Et = pE.tile([P, C], f32r, tag="E", name=f"E_{b}_{mc}")
                S_sb = psm.tile([P, 1], f32, tag="S", name=f"S_{b}_{mc}")
                nc.scalar.activation(Et[:], E_ps[mc][:], FT.Exp,
                                     bias=m_sb[:], scale=-1.0, accum_out=S_sb[:])
                rSt = psm.tile([P, 1], f32, tag="rS", name=f"rS_{b}_{mc}")
                nc.vector.reciprocal(rSt[:], S_sb[:])
                E_sb.append(Et)
                rS.append(rSt)
                yield

            # ---- phase C2: ET = E~^T (for the SE pooled-out term) ----
            ET = []
            for db in range(CB):
                et_ps = ptps.tile([P, C], f32r, tag="tps", name=f"et_{b}_{db}")
                for cb in range(CB):
                    nc.tensor.transpose(
                        et_ps[:, cb * P:(cb + 1) * P],
                        E_sb[cb][:, db * P:(db + 1) * P], identr[:])
                ETt = pET.tile([P, C], f32r, tag="ET", name=f"ET_{b}_{db}")
                if db % 2 == 0:
                    nc.scalar.activation(ETt[:], et_ps[:], FT.Copy)
                else:
                    nc.vector.tensor_copy(ETt[:], et_ps[:])
                ET.append(ETt)
                yield

            # ---- phase C3: SE gate ----
            pout = []
            for cb in range(CB):
                pp = pops.tile([P, 2], f32, tag="ops", name=f"pp_{b}_{cb}")
                for db in range(CB):
                    nc.tensor.matmul(pp[:], ET[db][:, cb * P:(cb + 1) * P],
                                     px[db][:], start=(db == 0),
                                     stop=(db == CB - 1))
                pot = psm.tile([P, 2], f32r, tag="pout", name=f"pout_{b}_{cb}")
                with nc.allow_low_precision(reason="SE gate pooled term"):
                    nc.vector.tensor_scalar(out=pot[:], in0=pp[:],
                                            scalar1=rS[cb][:], scalar2=None,
                                            op0=ALU.mult)
                pout.append(pot)
                yield

            h_ps = pops.tile([CR, 2], f32, tag="ops", name=f"h_ps_{b}")
            rhs_blocks = px + pout
            for kb in range(2 * C // P):
                nc.tensor.matmul(h_ps[:], w1tr[:, kb, :], rhs_blocks[kb][:],
                                 start=(kb == 0), stop=(kb == 2 * C // P - 1))
            h_sb = psm.tile([CR, 2], f32r, tag="h", name=f"h_{b}")
            with nc.allow_low_precision(reason="SE hidden"):
                nc.scalar.activation(h_sb[:], h_ps[:], FT.Relu,
                                     bias=b1_sb[:], scale=1.0)
            yield

            se, g = [], []
            for cb in range(CB):
                z_ps = pops.tile([P, 2], f32, tag="ops", name=f"z_ps_{b}_{cb}")
                nc.tensor.matmul(z_ps[:], w2tr[:, cb * P:(cb + 1) * P], h_sb[:],
                                 start=True, stop=True)
                # sigmoid(z + b2) = 1 / (1 + exp(-z - b2)); b2n = -b2
                en = psm.tile([P, 1], f32, tag="en", name=f"en_{b}_{cb}")
                nc.scalar.activation(en[:], z_ps[:, 0:1], FT.Exp,
                                     bias=b2n_sb[:, cb:cb + 1], scale=-1.0)
                den = psm.tile([P, 1], f32, tag="den", name=f"den_{b}_{cb}")
                nc.vector.tensor_scalar_add(den[:], en[:], 1.0)
                set_ = psm.tile([P, 1], f32, tag="se", name=f"se_{b}_{cb}")
                nc.vector.reciprocal(set_[:], den[:])
                onems = psm.tile([P, 1], f32, tag="onems", name=f"om_{b}_{cb}")
                nc.vector.tensor_scalar(out=onems[:], in0=set_[:], scalar1=-1.0,
                                        scalar2=1.0, op0=ALU.mult, op1=ALU.add)
                gt = psm.tile([P, 1], f32, tag="g", name=f"g_{b}_{cb}")
                nc.vector.tensor_mul(gt[:], onems[:], rS[cb][:])
                se.append(set_)
                g.append(gt)
                yield

            # ---- phase C4: A = g*E~ + diag(se); ATg = A^T ----
            Eg, dg = [], []
            for cb in range(CB):
                Egt = pEg.tile([P, C], f32r, tag="Eg", name=f"Eg_{b}_{cb}")
                nc.vector.tensor_scalar(out=Egt[:], in0=E_sb[cb][:],
                                        scalar1=g[cb][:], scalar2=None,
                                        op0=ALU.mult)
                Eg.append(Egt)
                dgt = psm.tile([P, P], f32r, tag="dg", name=f"dg_{b}_{cb}")
                nc.vector.tensor_scalar(out=dgt[:], in0=ident[:],
                                        scalar1=se[cb][:], scalar2=None,
                                        op0=ALU.mult)
                dg.append(dgt)
            ATg = []
            for db in range(CB):
                at_ps = ptps.tile([P, C], f32r, tag="tps", name=f"at_{b}_{db}")
                for i in range(CB):
                    nc.tensor.matmul(
                        at_ps[:, i * P:(i + 1) * P],
                        Eg[i][:, db * P:(db + 1) * P], identr[:],
                        is_transpose=True, start=(i == 0), stop=(i == CB - 1))
                At = pA.tile([P, C], f32r, tag="ATg", name=f"ATg_{b}_{db}")
                if db % 2 == 0:
                    nc.scalar.activation(At[:], at_ps[:], FT.Copy)
                else:
                    nc.vector.tensor_copy(At[:], at_ps[:])
                nc.vector.tensor_add(At[:, db * P:(db + 1) * P],
                                     At[:, db * P:(db + 1) * P], dg[db][:])
                ATg.append(At)
                yield
            out.append(ATg)

        def phaseD_gen(b, ATg, deep_psum=False):
            """Generator: yields after each (nb, cb) group (32 yields)."""
            q = qr[b]
            i = 0
            for nb in range(NB):
                for cb in range(CB):
                    # the last batch's phase D has the whole PSUM to itself:
                    # alternate over the energy banks too for a deeper ring
                    tag = "ops"
                    if deep_psum and i % 2 == 1:
                        tag = "eps"
                    o_ps = pops.tile([P, 512], f32, tag=tag,
                                     name=f"o_ps_{b}_{nb}_{cb}") \
                        if tag == "ops" else \
                        peps.tile([P, 512], f32, tag=tag,
                                  name=f"o_ps_{b}_{nb}_{cb}")
                    for db in range(CB):
                        nc.tensor.matmul(
                            o_ps[:], ATg[db][:, cb * P:(cb + 1) * P],
                            q[db][:, nb * 512:(nb + 1) * 512],
                            start=(db == 0), stop=(db == CB - 1))
                    f_ = pbl.tile([P, 512], f32, tag="f",
                                  name=f"f_{b}_{nb}_{cb}")
                    if cb % 2 == 0:
                        nc.scalar.activation(f_[:], o_ps[:], FT.Copy)
                    else:
                        nc.vector.tensor_copy(f_[:], o_ps[:])
                    nc.sync.dma_start(
                        y_d[b, cb * P:(cb + 1) * P, nb * 512:(nb + 1) * 512],
                        f_[:])
                    i += 1
                    yield

        # ---- schedule ----
        # head: ident, first q chunks, then weights, then remaining chunks
        pairs0 = make_load_pairs(0)
        for i, p in enumerate(pairs0[:CB]):
            p(i)
        weights = emit_weight_loads()
        for i, p in enumerate(pairs0[CB:], start=CB):
            p(i)
        px0 = emit_px(0)
        pairs1 = make_load_pairs(1) if BP > 1 else None

        # B0 with b1's loads sprinkled in
        for _ in phaseB_gen(0, sprinkle=pairs1):
            pass

        aout0, aout1 = [], []
        if BP == 1:
            for _ in phaseC_gen(0, px0, weights, aout0):
                pass
            for _ in phaseD_gen(0, aout0[0], deep_psum=True):
                pass
        else:
            px1 = emit_px(1)
            gB1 = phaseB_gen(1)
            # C0 woven with B1 (B1's PE work fills C0's dependency stalls)
            for _ in phaseC_gen(0, px0, weights, aout0):
                next(gB1, None)
            # B1 remainder woven with D0
            gD0 = phaseD_gen(0, aout0[0])
            while next(gB1, "end") != "end":
                next(gD0, None)
            # C1 woven with D0 remainder
            for _ in phaseC_gen(1, px1, weights, aout1):
                next(gD0, None)
            for _ in gD0:
                pass
            for _ in phaseD_gen(1, aout1[0], deep_psum=True):
                pass


def _get_program():
    with _lock:
        if "nc" not in _cached:
            _cached["nc"] = _build()
    return _cached["nc"]


def _prep_in_maps(x, w1, b1, w2, b2):
    x = np.ascontiguousarray(np.asarray(x, dtype=np.float32)).reshape(B, C, N)
    w1 = np.asarray(w1, dtype=np.float32)
    b1 = np.asarray(b1, dtype=np.float32)
    w2 = np.asarray(w2, dtype=np.float32)
    b2 = np.asarray(b2, dtype=np.float32)

    w1t = (np.ascontiguousarray(w1.T) / np.float32(N)).astype(np.float32)
    w2t = np.ascontiguousarray(w2.T)
    b1c = np.ascontiguousarray(b1.reshape(CR, 1))
    b2n = np.ascontiguousarray(-b2.reshape(C, 1))
    ident = np.eye(P, dtype=np.float32)

    in_maps = []
    for c in range(NCORES):
        in_maps.append({
            "x": np.ascontiguousarray(x[c * BP:(c + 1) * BP]),
            "w1t": w1t,
            "b1": b1c,
            "w2t": w2t,
            "b2n": b2n,
            "ident": ident,
        })
    return in_maps


def run(x, w1, b1, w2, b2, trace=False):
    nc = _get_program()
    in_maps = _prep_in_maps(x, w1, b1, w2, b2)
    res = run_bass_kernel_spmd(nc, in_maps, core_ids=list(range(NCORES)),
                               trace=trace)
    y = np.concatenate([res.results[c]["y"][None] for c in range(NCORES)], axis=0)
    y = y.reshape(B, C, H, W).astype(np.float32)
    return y, res


def kernel(x, w1, b1, w2, b2):
    y, _ = run(x, w1, b1, w2, b2, trace=False)
    return y


# revision 38
# speedup vs baseline: 15.0966x; 15.0966x over previous
"""Trainium2 Bass kernel for nn_CA_Module (DANet CAM + SE gate).

Reference math (per batch item b):
    q = x[b].reshape(C, N)                         # C=512, N=4096
    energy = q @ q.T                               # [C, C]
    att = softmax(max_row(energy) - energy)        # == softmax(-energy) rows
    out = att @ q                                  # [C, N]
    pooled = concat([mean_n x, mean_n out])        # [2C]
    hidden = relu(pooled @ w1.T + b1)              # [CR]
    se = sigmoid(hidden @ w2.T + b2)               # [C]
    y = se * x + (1 - se) * out

Sharding: data-parallel over B=16 across 8 cores (2 batch items/core).

Device implementation highlights:
  - One f32r copy of q serves everything: DMA lands raw f32 bits in the q_r
    tile, an in-place Copy rounds to f32r (satisfying the fp32r-producer
    rule) and its accum_out gives the pooled-x sums for free.
  - energy is computed upper-block-triangle only (it is symmetric); the
    missing blocks are mirrored with PE transposes into the same PSUM banks.
  - att row c: exp(min_row(energy)[c] - energy[c,:]) / S[c]; S comes free
    from the exp's accum_out.
  - final blend folds into the second matmul: y = A @ q with
    A = ((1-se)/S) * E~ + diag(se), so phase D is pure matmul + copy-out.
  - mean_n out is not reduced from the big out tensor: sum_n out_raw =
    E~ @ (sum_n q), a [512,512]@[512,2] matmul against the pooled-x sums.
  - sigmoid is exp(-z) -> +1 -> reciprocal so the ACT engine stays on the
    exp table set the whole kernel (table switches cost ~2.7us).
  - w1t is pre-scaled by 1/N on the host so pooled sums need no extra scale.
"""
import threading
import numpy as np

import concourse.bass as bass
import concourse.tile as tile
from concourse import bacc, mybir, masks
from concourse.bass_utils import run_bass_kernel_spmd

B, C, H, W = 16, 512, 64, 64
N = H * W                 # 4096
NCORES = 8
BP = B // NCORES          # batch items per core
CR = C // 8               # 64
P = 128                   # partitions
CB = C // P               # 4 c-blocks
NK = N // P               # 32 n-blocks of 128
NB = N // 512             # 8 n-chunks of 512
QCH = 8                   # q DMA/cast chunks per c-block
QCW = N // QCH            # chunk width (512)

f32 = mybir.dt.float32
f32r = mybir.dt.float32r
FT = mybir.ActivationFunctionType
ALU = mybir.AluOpType
AX = mybir.AxisListType

_lock = threading.Lock()
_cached = {}


def _build():
    nc = bacc.Bacc("TRN2", target_bir_lowering=False, debug=False,
                   num_devices=NCORES)

    x_d = nc.dram_tensor("x", [BP, C, N], f32, kind="ExternalInput").ap()
    w1t_d = nc.dram_tensor("w1t", [2 * C, CR], f32, kind="ExternalInput").ap()
    b1_d = nc.dram_tensor("b1", [CR, 1], f32, kind="ExternalInput").ap()
    w2t_d = nc.dram_tensor("w2t", [CR, C], f32, kind="ExternalInput").ap()
    b2n_d = nc.dram_tensor("b2n", [C, 1], f32, kind="ExternalInput").ap()
    ident_d = nc.dram_tensor("ident", [P, P], f32, kind="ExternalInput").ap()
    y_d = nc.dram_tensor("y", [BP, C, N], f32, kind="ExternalOutput").ap()

    with tile.TileContext(nc) as tc:
        _emit(nc, tc, x_d, w1t_d, b1_d, w2t_d, b2n_d, ident_d, y_d)
    nc.compile()
    return nc


def _emit(nc, tc, x_d, w1t_d, b1_d, w2t_d, b2n_d, ident_d, y_d):
    from contextlib import ExitStack
    ctx = ExitStack()
    with ctx:
        consts = ctx.enter_context(tc.tile_pool(name="consts", bufs=1))
        pq = ctx.enter_context(tc.tile_pool(name="pq", bufs=2 * CB))
        pst = ctx.enter_context(tc.tile_pool(name="pst", bufs=3))
        pqT = ctx.enter_context(tc.tile_pool(name="pqT", bufs=3))
        pE = ctx.enter_context(tc.tile_pool(name="pE", bufs=4))
        pET = ctx.enter_context(tc.tile_pool(name="pET", bufs=4))
        pA = ctx.enter_context(tc.tile_pool(name="pA", bufs=4))
        pEg = ctx.enter_context(tc.tile_pool(name="pEg", bufs=4))
        pmir = ctx.enter_context(tc.tile_pool(name="pmir", bufs=1))
        pbl = ctx.enter_context(tc.tile_pool(name="pbl", bufs=6))
        psm = ctx.enter_context(tc.tile_pool(name="psm", bufs=8))
        # PSUM: eps(4) + tps(2) + ops(2) = 8 banks
        peps = ctx.enter_context(
            tc.tile_pool(name="peps", bufs=4, space=bass.MemorySpace.PSUM))
        ptps = ctx.enter_context(
            tc.tile_pool(name="ptps", bufs=2, space=bass.MemorySpace.PSUM))
        pops = ctx.enter_context(
            tc.tile_pool(name="pops", bufs=2, space=bass.MemorySpace.PSUM))

        # ---- constants (weights DMA'd later, after the first q chunks) ----
        ident = consts.tile([P, P], f32, tag="ident")
        nc.sync.dma_start(ident[:], ident_d[:])
        identr = consts.tile([P, P], f32r, tag="identr")
        nc.vector.tensor_copy(identr[:], ident[:])

        def emit_weight_loads():
            w1t_sb = consts.tile([P, 2 * C // P, CR], f32, tag="w1t",
                                 name="w1t_sb")
            nc.sync.dma_start(w1t_sb[:],
                              w1t_d.rearrange("(kb p) j -> p kb j", p=P))
            w1tr = consts.tile([P, 2 * C // P, CR], f32r, tag="w1tr",
                               name="w1tr")
            nc.vector.tensor_copy(w1tr[:], w1t_sb[:])

            w2t_sb = consts.tile([CR, C], f32, tag="w2t", name="w2t_sb")
            nc.sync.dma_start(w2t_sb[:], w2t_d[:])
            w2tr = consts.tile([CR, C], f32r, tag="w2tr", name="w2tr")
            nc.vector.tensor_copy(w2tr[:], w2t_sb[:])

            b1_sb = consts.tile([CR, 1], f32, tag="b1", name="b1_sb")
            nc.sync.dma_start(b1_sb[:], b1_d[:])
            b2n_sb = consts.tile([P, CB], f32, tag="b2n", name="b2n_sb")
            nc.sync.dma_start(b2n_sb[:],
                              b2n_d.rearrange("(cb p) one -> p (cb one)", p=P))
            return w1tr, w2tr, b1_sb, b2n_sb

        # ---- per-batch state ----
        qr = {}      # b -> [CB] tiles [P, N] f32r
        pxacc = {}   # b -> [CB] accumulator tiles [P, QCH] f32

        def make_load_pairs(b):
            """Closures, one per (chunk, cb): DMA x chunk -> staging, then
            cast staging -> q_r chunk (accumulating pooled-x row sums)."""
            tiles = []
            for cb in range(CB):
                t = pq.tile([P, N], f32r, tag="q", name=f"q_{b}_{cb}")
                tiles.append(t)
            qr[b] = tiles
            pxacc[b] = [psm.tile([P, QCH], f32, tag="pxacc",
                                 name=f"pxacc_{b}_{cb}") for cb in range(CB)]

            def pair(cb, ch):
                def go(i):
                    st = pst.tile([P, QCW], f32, tag="qst",
                                  name=f"qst_{b}_{cb}_{ch}")
                    nc.sync.dma_start(
                        st[:],
                        x_d[b, cb * P:(cb + 1) * P, ch * QCW:(ch + 1) * QCW])
                    dst = tiles[cb][:, ch * QCW:(ch + 1) * QCW]
                    acc = pxacc[b][cb][:, ch:ch + 1]
                    with nc.allow_low_precision(reason="f32r round of q"):
                        nc.vector.tensor_scalar(
                            out=dst, in0=st[:], scalar1=1.0, scalar2=0.0,
                            op0=ALU.mult, op1=ALU.add, accum_out=acc)
                return go

            return [pair(cb, ch) for ch in range(QCH) for cb in range(CB)]

        def emit_px(b):
            px = []
            for cb in range(CB):
                pxt = psm.tile([P, 2], f32r, tag="px", name=f"px_{b}_{cb}")
                with nc.allow_low_precision(reason="pooled sums feed SE gate"):
                    nc.vector.tensor_reduce(pxt[:, 0:1], pxacc[b][cb][:],
                                            axis=AX.X, op=ALU.add)
                    nc.vector.tensor_copy(pxt[:, 1:2], pxt[:, 0:1])
                px.append(pxt)
            return px

        E_ps_of = {}

        def phaseB_gen(b, sprinkle=None):
            """Generator: yields after each k iteration (32 yields)."""
            q = qr[b]
            E_ps = [peps.tile([P, C], f32, tag="eps", name=f"E_ps_{b}_{i}")
                    for i in range(CB)]
            E_ps_of[b] = E_ps

            def emit_transpose_block(k):
                t_ps = ptps.tile([P, C], f32r, tag="tps", name=f"t_ps_{b}_{k}")
                for cb in range(CB):
                    nc.tensor.transpose(
                        t_ps[:, cb * P:(cb + 1) * P],
                        q[cb][:, k * P:(k + 1) * P], identr[:])
                qTt = pqT.tile([P, C], f32r, tag="qT", name=f"qT_{b}_{k}")
                nc.scalar.activation(qTt[:], t_ps[:], FT.Copy)
                return qTt

            prev_qT = emit_transpose_block(0)
            for k in range(NK):
                nxt_qT = emit_transpose_block(k + 1) if k + 1 < NK else None
                for mc in range(CB):
                    nc.tensor.matmul(
                        E_ps[mc][:, mc * P:C],
                        prev_qT[:, mc * P:(mc + 1) * P],
                        prev_qT[:, mc * P:C],
                        start=(k == 0), stop=(k == NK - 1))
                if sprinkle is not None:
                    for j, fn in enumerate(sprinkle.get(k, ())):
                        fn(k + j)
                prev_qT = nxt_qT
                yield

        def phaseC_gen(b, px, weights, out):
            w1tr, w2tr, b1_sb, b2n_sb = weights
            E_ps = E_ps_of[b]
            # mirror lower-triangle blocks: E[md, mc] = E[mc, md]^T
            for mc in range(CB):
                for md in range(mc + 1, CB):
                    mt = pmir.tile([P, P], f32, tag="mir",
                                   name=f"mir_{b}_{mc}_{md}")
                    nc.vector.tensor_copy(mt[:], E_ps[mc][:, md * P:(md + 1) * P])
                    nc.tensor.matmul(
                        E_ps[md][:, mc * P:(mc + 1) * P], mt[:], ident[:],
                        is_transpose=True, start=False, stop=True)
                yield
            # ---- phase C: softmax pieces ----
            E_sb, rS = [], []
            for mc in range(CB):
                m_sb = psm.tile([P, 1], f32, tag="m", name=f"m_{b}_{mc}")
                nc.vector.tensor_reduce(m_sb[:], E_ps[mc][:], axis=AX.X,
                                        op=ALU.min)
                Et = pE.tile([P, C], f32r, tag="E", name=f"E_{b}_{mc}")
                S_sb = psm.tile([P, 1], f32, tag="S", name=f"S_{b}_{mc}")
                nc.scalar.activation(Et[:], E_ps[mc][:], FT.Exp,
                                     bias=m_sb[:], scale=-1.0, accum_out=S_sb[:])
                rSt = psm.tile([P, 1], f32, tag="rS", name=f"rS_{b}_{mc}")
                nc.vector.reciprocal(rSt[:], S_sb[:])
                E_sb.append(Et)
                rS.append(rSt)
                yield

            # ---- phase C2: ET = E~^T (for the SE pooled-out term) ----
            ET = []
            for db in range(CB):
                et_ps = ptps.tile([P, C], f32r, tag="tps", name=f"et_{b}_{db}")
                for cb in range(CB):
                    nc.tensor.transpose(
                        et_ps[:, cb * P:(cb + 1) * P],
                        E_sb[cb][:, db * P:(db + 1) * P], identr[:])
                ETt = pET.tile([P, C], f32r, tag="ET", name=f"ET_{b}_{db}")
                if db % 2 == 0:
                    nc.scalar.activation(ETt[:], et_ps[:], FT.Copy)
                else:
                    nc.vector.tensor_copy(ETt[:], et_ps[:])
                ET.append(ETt)
                yield

            # ---- phase C3: SE gate ----
            pout = []
            for cb in range(CB):
                pp = pops.tile([P, 2], f32, tag="ops", name=f"pp_{b}_{cb}")
                for db in range(CB):
                    nc.tensor.matmul(pp[:], ET[db][:, cb * P:(cb + 1) * P],
                                     px[db][:], start=(db == 0),
                                     stop=(db == CB - 1))
                pot = psm.tile([P, 2], f32r, tag="pout", name=f"pout_{b}_{cb}")
                with nc.allow_low_precision(reason="SE gate pooled term"):
                    nc.vector.tensor_scalar(out=pot[:], in0=pp[:],
                                            scalar1=rS[cb][:], scalar2=None,
                                            op0=ALU.mult)
                pout.append(pot)
                yield

            h_ps = pops.tile([CR, 2], f32, tag="ops", name=f"h_ps_{b}")
            rhs_blocks = px + pout
            for kb in range(2 * C // P):
                nc.tensor.matmul(h_ps[:], w1tr[:, kb, :], rhs_blocks[kb][:],
                                 start=(kb == 0), stop=(kb == 2 * C // P - 1))
            h_sb = psm.tile([CR, 2], f32r, tag="h", name=f"h_{b}")
            with nc.allow_low_precision(reason="SE hidden"):
                nc.scalar.activation(h_sb[:], h_ps[:], FT.Relu,
                                     bias=b1_sb[:], scale=1.0)
            yield

            se, g = [], []
            for cb in range(CB):
                z_ps = pops.tile([P, 2], f32, tag="ops", name=f"z_ps_{b}_{cb}")
                nc.tensor.matmul(z_ps[:], w2tr[:, cb * P:(cb + 1) * P], h_sb[:],
                                 start=True, stop=True)
                # sigmoid(z + b2) = 1 / (1 + exp(-z - b2)); b2n = -b2
                en = psm.tile([P, 1], f32, tag="en", name=f"en_{b}_{cb}")
                nc.scalar.activation(en[:], z_ps[:, 0:1], FT.Exp,
                                     bias=b2n_sb[:, cb:cb + 1], scale=-1.0)
                den = psm.tile([P, 1], f32, tag="den", name=f"den_{b}_{cb}")
                nc.vector.tensor_scalar_add(den[:], en[:], 1.0)
                set_ = psm.tile([P, 1], f32, tag="se", name=f"se_{b}_{cb}")
                nc.vector.reciprocal(set_[:], den[:])
                onems = psm.tile([P, 1], f32, tag="onems", name=f"om_{b}_{cb}")
                nc.vector.tensor_scalar(out=onems[:], in0=set_[:], scalar1=-1.0,
                                        scalar2=1.0, op0=ALU.mult, op1=ALU.add)
                gt = psm.tile([P, 1], f32, tag="g", name=f"g_{b}_{cb}")
                nc.vector.tensor_mul(gt[:], onems[:], rS[cb][:])
                se.append(set_)
                g.append(gt)
                yield

            # ---- phase C4: A = g*E~ + diag(se); ATg = A^T ----
            Eg, dg = [], []
            for cb in range(CB):
                Egt = pEg.tile([P, C], f32r, tag="Eg", name=f"Eg_{b}_{cb}")
                nc.vector.tensor_scalar(out=Egt[:], in0=E_sb[cb][:],
                                        scalar1=g[cb][:], scalar2=None,
                                        op0=ALU.mult)
                Eg.append(Egt)
                dgt = psm.tile([P, P], f32r, tag="dg", name=f"dg_{b}_{cb}")
                nc.vector.tensor_scalar(out=dgt[:], in0=ident[:],
                                        scalar1=se[cb][:], scalar2=None,
                                        op0=ALU.mult)
                dg.append(dgt)
            ATg = []
            for db in range(CB):
                at_ps = ptps.tile([P, C], f32r, tag="tps", name=f"at_{b}_{db}")
                for i in range(CB):
                    nc.tensor.matmul(
                        at_ps[:, i * P:(i + 1) * P],
                        Eg[i][:, db * P:(db + 1) * P], identr[:],
                        is_transpose=True, start=(i == 0), stop=(i == CB - 1))
                At = pA.tile([P, C], f32r, tag="ATg", name=f"ATg_{b}_{db}")
                if db % 2 == 0:
                    nc.scalar.activation(At[:], at_ps[:], FT.Copy)
                else:
                    nc.vector.tensor_copy(At[:], at_ps[:])
                nc.vector.tensor_add(At[:, db * P:(db + 1) * P],
                                     At[:, db * P:(db + 1) * P], dg[db][:])
                ATg.append(At)
                yield
            out.append(ATg)

        def phaseD_gen(b, ATg, deep_psum=False):
            """Generator: yields after each (nb, cb) group (32 yields)."""
            q = qr[b]
            i = 0
            for nb in range(NB):
                for cb in range(CB):
                    # the last batch's phase D has the whole PSUM to itself:
                    # alternate over the energy banks too for a deeper ring
                    tag = "ops"
                    if deep_psum and i % 2 == 1:
                        tag = "eps"
                    o_ps = pops.tile([P, 512], f32, tag=tag,
                                     name=f"o_ps_{b}_{nb}_{cb}") \
                        if tag == "ops" else \
                        peps.tile([P, 512], f32, tag=tag,
                                  name=f"o_ps_{b}_{nb}_{cb}")
                    for db in range(CB):
                        nc.tensor.matmul(
                            o_ps[:], ATg[db][:, cb * P:(cb + 1) * P],
                            q[db][:, nb * 512:(nb + 1) * 512],
                            start=(db == 0), stop=(db == CB - 1))
                    f_ = pbl.tile([P, 512], f32, tag="f",
                                  name=f"f_{b}_{nb}_{cb}")
                    if cb % 2 == 0:
                        nc.scalar.activation(f_[:], o_ps[:], FT.Copy)
                    else:
                        nc.vector.tensor_copy(f_[:], o_ps[:])
                    nc.sync.dma_start(
                        y_d[b, cb * P:(cb + 1) * P, nb * 512:(nb + 1) * 512],
                        f_[:])
                    i += 1
                    yield

        # ---- schedule ----
        # head: ident, first q chunks, then weights, then remaining chunks
        pairs0 = make_load_pairs(0)
        for i, p in enumerate(pairs0[:CB]):
            p(i)
        weights = emit_weight_loads()
        for i, p in enumerate(pairs0[CB:], start=CB):
            p(i)
        px0 = emit_px(0)
        sprinkle1 = None
        if BP > 1:
            pairs1 = make_load_pairs(1)
            sprinkle1 = {i: [pairs1[i]] for i in range(len(pairs1))}

        # B0 with b1's loads sprinkled in
        for _ in phaseB_gen(0, sprinkle=sprinkle1):
            pass

        aout0, aout1 = [], []
        if BP == 1:
            for _ in phaseC_gen(0, px0, weights, aout0):
                pass
            for _ in phaseD_gen(0, aout0[0], deep_psum=True):
                pass
        else:
            px1 = emit_px(1)
            gB1 = phaseB_gen(1)
            # C0 woven with B1 (B1's PE work fills C0's dependency stalls)
            for _ in phaseC_gen(0, px0, weights, aout0):
                next(gB1, None)
            # B1 remainder woven with D0
            gD0 = phaseD_gen(0, aout0[0])
            while next(gB1, "end") != "end":
                next(gD0, None)
            # C1 woven with D0 remainder
            for _ in phaseC_gen(1, px1, weights, aout1):
                next(gD0, None)
            for _ in gD0:
                pass
            for _ in phaseD_gen(1, aout1[0], deep_psum=True):
                pass


def _get_program():
    with _lock:
        if "nc" not in _cached:
            _cached["nc"] = _build()
    return _cached["nc"]


def _prep_in_maps(x, w1, b1, w2, b2):
    x = np.ascontiguousarray(np.asarray(x, dtype=np.float32)).reshape(B, C, N)
    w1 = np.asarray(w1, dtype=np.float32)
    b1 = np.asarray(b1, dtype=np.float32)
    w2 = np.asarray(w2, dtype=np.float32)
    b2 = np.asarray(b2, dtype=np.float32)

    w1t = (np.ascontiguousarray(w1.T) / np.float32(N)).astype(np.float32)
    w2t = np.ascontiguousarray(w2.T)
    b1c = np.ascontiguousarray(b1.reshape(CR, 1))
    b2n = np.ascontiguousarray(-b2.reshape(C, 1))
    ident = np.eye(P, dtype=np.float32)

    in_maps = []
    for c in range(NCORES):
        in_maps.append({
            "x": np.ascontiguousarray(x[c * BP:(c + 1) * BP]),
            "w1t": w1t,
            "b1": b1c,
            "w2t": w2t,
            "b2n": b2n,
            "ident": ident,
        })
    return in_maps


def run(x, w1, b1, w2, b2, trace=False):
    nc = _get_program()
    in_maps = _prep_in_maps(x, w1, b1, w2, b2)
    res = run_bass_kernel_spmd(nc, in_maps, core_ids=list(range(NCORES)),
                               trace=trace)
    y = np.concatenate([res.results[c]["y"][None] for c in range(NCORES)], axis=0)
    y = y.reshape(B, C, H, W).astype(np.float32)
    return y, res


def kernel(x, w1, b1, w2, b2):
    y, _ = run(x, w1, b1, w2, b2, trace=False)
    return y


# revision 44
# speedup vs baseline: 883665.1964x; 58533.9271x over previous
"""Trainium2 Bass kernel for nn_CA_Module (DANet CAM + SE gate).

Reference math (per batch item b):
    q = x[b].reshape(C, N)                         # C=512, N=4096
    energy = q @ q.T                               # [C, C]
    att = softmax(max_row(energy) - energy)        # == softmax(-energy) rows
    out = att @ q                                  # [C, N]
    pooled = concat([mean_n x, mean_n out])        # [2C]
    hidden = relu(pooled @ w1.T + b1)              # [CR]
    se = sigmoid(hidden @ w2.T + b2)               # [C]
    y = se * x + (1 - se) * out

Sharding: data-parallel over B=16 across 8 cores (2 batch items/core).

Device implementation highlights:
  - One f32r copy of q serves everything: DMA lands raw f32 bits in the q_r
    tile, an in-place Copy rounds to f32r (satisfying the fp32r-producer
    rule) and its accum_out gives the pooled-x sums for free.
  - energy is computed upper-block-triangle only (it is symmetric); the
    missing blocks are mirrored with PE transposes into the same PSUM banks.
  - att row c: exp(min_row(energy)[c] - energy[c,:]) / S[c]; S comes free
    from the exp's accum_out.
  - final blend folds into the second matmul: y = A @ q with
    A = ((1-se)/S) * E~ + diag(se), so phase D is pure matmul + copy-out.
  - mean_n out is not reduced from the big out tensor: sum_n out_raw =
    E~ @ (sum_n q), a [512,512]@[512,2] matmul against the pooled-x sums.
  - sigmoid is exp(-z) -> +1 -> reciprocal so the ACT engine stays on the
    exp table set the whole kernel (table switches cost ~2.7us).
  - w1t is pre-scaled by 1/N on the host so pooled sums need no extra scale.
"""
import threading
import numpy as np

import concourse.bass as bass
import concourse.tile as tile
from concourse import bacc, mybir, masks
from concourse.bass_utils import run_bass_kernel_spmd

B, C, H, W = 16, 512, 64, 64
N = H * W                 # 4096
NCORES = 8
BP = B // NCORES          # batch items per core
CR = C // 8               # 64
P = 128                   # partitions
CB = C // P               # 4 c-blocks
NK = N // P               # 32 n-blocks of 128
NB = N // 512             # 8 n-chunks of 512
QCH = 8                   # q DMA/cast chunks per c-block
QCW = N // QCH            # chunk width (512)

f32 = mybir.dt.float32
f32r = mybir.dt.float32r
FT = mybir.ActivationFunctionType
ALU = mybir.AluOpType
AX = mybir.AxisListType

_lock = threading.Lock()
_cached = {}


def _build():
    nc = bacc.Bacc("TRN2", target_bir_lowering=False, debug=False,
                   num_devices=NCORES)

    x_d = nc.dram_tensor("x", [BP, C, N], f32, kind="ExternalInput").ap()
    w1t_d = nc.dram_tensor("w1t", [2 * C, CR], f32, kind="ExternalInput").ap()
    b1_d = nc.dram_tensor("b1", [CR, 1], f32, kind="ExternalInput").ap()
    w2t_d = nc.dram_tensor("w2t", [CR, C], f32, kind="ExternalInput").ap()
    b2n_d = nc.dram_tensor("b2n", [C, 1], f32, kind="ExternalInput").ap()
    ident_d = nc.dram_tensor("ident", [P, P], f32, kind="ExternalInput").ap()
    y_d = nc.dram_tensor("y", [BP, C, N], f32, kind="ExternalOutput").ap()

    with tile.TileContext(nc) as tc:
        _emit(nc, tc, x_d, w1t_d, b1_d, w2t_d, b2n_d, ident_d, y_d)
    nc.compile()
    return nc


def _emit(nc, tc, x_d, w1t_d, b1_d, w2t_d, b2n_d, ident_d, y_d):
    from contextlib import ExitStack
    ctx = ExitStack()
    with ctx:
        consts = ctx.enter_context(tc.tile_pool(name="consts", bufs=1))
        pq = ctx.enter_context(tc.tile_pool(name="pq", bufs=2 * CB))
        pst = ctx.enter_context(tc.tile_pool(name="pst", bufs=3))
        pqT = ctx.enter_context(tc.tile_pool(name="pqT", bufs=3))
        pE = ctx.enter_context(tc.tile_pool(name="pE", bufs=4))
        pET = ctx.enter_context(tc.tile_pool(name="pET", bufs=4))
        pA = ctx.enter_context(tc.tile_pool(name="pA", bufs=4))
        pEg = ctx.enter_context(tc.tile_pool(name="pEg", bufs=4))
        pmir = ctx.enter_context(tc.tile_pool(name="pmir", bufs=1))
        pbl = ctx.enter_context(tc.tile_pool(name="pbl", bufs=6))
        psm = ctx.enter_context(tc.tile_pool(name="psm", bufs=8))
        # PSUM: eps(4) + tps(2) + ops(2) = 8 banks
        peps = ctx.enter_context(
            tc.tile_pool(name="peps", bufs=4, space=bass.MemorySpace.PSUM))
        ptps = ctx.enter_context(
            tc.tile_pool(name="ptps", bufs=2, space=bass.MemorySpace.PSUM))
        pops = ctx.enter_context(
            tc.tile_pool(name="pops", bufs=2, space=bass.MemorySpace.PSUM))

        # ---- constants (weights DMA'd later, after the first q chunks) ----
        ident = consts.tile([P, P], f32, tag="ident")
        nc.sync.dma_start(ident[:], ident_d[:])
        identr = consts.tile([P, P], f32r, tag="identr")
        nc.vector.tensor_copy(identr[:], ident[:])

        def emit_weight_loads():
            w1t_sb = consts.tile([P, 2 * C // P, CR], f32, tag="w1t",
                                 name="w1t_sb")
            nc.sync.dma_start(w1t_sb[:],
                              w1t_d.rearrange("(kb p) j -> p kb j", p=P))
            w1tr = consts.tile([P, 2 * C // P, CR], f32r, tag="w1tr",
                               name="w1tr")
            nc.vector.tensor_copy(w1tr[:], w1t_sb[:])

            w2t_sb = consts.tile([CR, C], f32, tag="w2t", name="w2t_sb")
            nc.sync.dma_start(w2t_sb[:], w2t_d[:])
            w2tr = consts.tile([CR, C], f32r, tag="w2tr", name="w2tr")
            nc.vector.tensor_copy(w2tr[:], w2t_sb[:])

            b1_sb = consts.tile([CR, 1], f32, tag="b1", name="b1_sb")
            nc.sync.dma_start(b1_sb[:], b1_d[:])
            b2n_sb = consts.tile([P, CB], f32, tag="b2n", name="b2n_sb")
            nc.sync.dma_start(b2n_sb[:],
                              b2n_d.rearrange("(cb p) one -> p (cb one)", p=P))
            return w1tr, w2tr, b1_sb, b2n_sb

        # ---- per-batch state ----
        qr = {}      # b -> [CB] tiles [P, N] f32r
        pxacc = {}   # b -> [CB] accumulator tiles [P, QCH] f32

        def make_load_pairs(b):
            """Closures, one per (chunk, cb): DMA x chunk -> staging, then
            cast staging -> q_r chunk (accumulating pooled-x row sums)."""
            tiles = []
            for cb in range(CB):
                t = pq.tile([P, N], f32r, tag="q", name=f"q_{b}_{cb}")
                tiles.append(t)
            qr[b] = tiles
            pxacc[b] = [psm.tile([P, QCH], f32, tag="pxacc",
                                 name=f"pxacc_{b}_{cb}") for cb in range(CB)]

            def pair(cb, ch):
                def go(i):
                    st = pst.tile([P, QCW], f32, tag="qst",
                                  name=f"qst_{b}_{cb}_{ch}")
                    nc.sync.dma_start(
                        st[:],
                        x_d[b, cb * P:(cb + 1) * P, ch * QCW:(ch + 1) * QCW])
                    dst = tiles[cb][:, ch * QCW:(ch + 1) * QCW]
                    acc = pxacc[b][cb][:, ch:ch + 1]
                    with nc.allow_low_precision(reason="f32r round of q"):
                        nc.vector.tensor_scalar(
                            out=dst, in0=st[:], scalar1=1.0, scalar2=0.0,
                            op0=ALU.mult, op1=ALU.add, accum_out=acc)
                return go

            return [pair(cb, ch) for ch in range(QCH) for cb in range(CB)]

        def emit_px(b):
            px = []
            for cb in range(CB):
                pxt = psm.tile([P, 2], f32r, tag="px", name=f"px_{b}_{cb}")
                with nc.allow_low_precision(reason="pooled sums feed SE gate"):
                    nc.vector.tensor_reduce(pxt[:, 0:1], pxacc[b][cb][:],
                                            axis=AX.X, op=ALU.add)
                    nc.vector.tensor_copy(pxt[:, 1:2], pxt[:, 0:1])
                px.append(pxt)
            return px

        E_ps_of = {}

        def phaseB_gen(b, sprinkle=None):
            """Generator: yields after each k iteration (32 yields)."""
            q = qr[b]
            E_ps = [peps.tile([P, C], f32, tag="eps", name=f"E_ps_{b}_{i}")
                    for i in range(CB)]
            E_ps_of[b] = E_ps

            def emit_transpose_block(k):
                t_ps = ptps.tile([P, C], f32r, tag="tps", name=f"t_ps_{b}_{k}")
                for cb in range(CB):
                    nc.tensor.transpose(
                        t_ps[:, cb * P:(cb + 1) * P],
                        q[cb][:, k * P:(k + 1) * P], identr[:])
                qTt = pqT.tile([P, C], f32r, tag="qT", name=f"qT_{b}_{k}")
                nc.scalar.activation(qTt[:], t_ps[:], FT.Copy)
                return qTt

            prev_qT = emit_transpose_block(0)
            for k in range(NK):
                nxt_qT = emit_transpose_block(k + 1) if k + 1 < NK else None
                for mc in range(CB):
                    nc.tensor.matmul(
                        E_ps[mc][:, mc * P:C],
                        prev_qT[:, mc * P:(mc + 1) * P],
                        prev_qT[:, mc * P:C],
                        start=(k == 0), stop=(k == NK - 1))
                if sprinkle is not None:
                    for j, fn in enumerate(sprinkle.get(k, ())):
                        fn(k + j)
                prev_qT = nxt_qT
                yield

        def phaseC_gen(b, px, weights, out):
            w1tr, w2tr, b1_sb, b2n_sb = weights
            E_ps = E_ps_of[b]
            # mirror lower-triangle blocks: E[md, mc] = E[mc, md]^T
            for mc in range(CB):
                for md in range(mc + 1, CB):
                    mt = pmir.tile([P, P], f32, tag="mir",
                                   name=f"mir_{b}_{mc}_{md}")
                    nc.vector.tensor_copy(mt[:], E_ps[mc][:, md * P:(md + 1) * P])
                    nc.tensor.matmul(
                        E_ps[md][:, mc * P:(mc + 1) * P], mt[:], ident[:],
                        is_transpose=True, start=False, stop=True)
                yield
            # ---- phase C: softmax pieces ----
            E_sb, rS = [], []
            for mc in range(CB):
                m_sb = psm.tile([P, 1], f32, tag="m", name=f"m_{b}_{mc}")
                nc.vector.tensor_reduce(m_sb[:], E_ps[mc][:], axis=AX.X,
                                        op=ALU.min)
                Et = pE.tile([P, C], f32r, tag="E", name=f"E_{b}_{mc}")
                S_sb = psm.tile([P, 1], f32, tag="S", name=f"S_{b}_{mc}")
                nc.scalar.activation(Et[:], E_ps[mc][:], FT.Exp,
                                     bias=m_sb[:], scale=-1.0, accum_out=S_sb[:])
                rSt = psm.tile([P, 1], f32, tag="rS", name=f"rS_{b}_{mc}")
                nc.vector.reciprocal(rSt[:], S_sb[:])
                E_sb.append(Et)
                rS.append(rSt)
                yield

            # ---- phase C2: ET = E~^T (for the SE pooled-out term) ----
            ET = []
            for db in range(CB):
                et_ps = ptps.tile([P, C], f32r, tag="tps", name=f"et_{b}_{db}")
                for cb in range(CB):
                    nc.tensor.transpose(
                        et_ps[:, cb * P:(cb + 1) * P],
                        E_sb[cb][:, db * P:(db + 1) * P], identr[:])
                ETt = pET.tile([P, C], f32r, tag="ET", name=f"ET_{b}_{db}")
                if db % 2 == 0:
                    nc.scalar.activation(ETt[:], et_ps[:], FT.Copy)
                else:
                    nc.vector.tensor_copy(ETt[:], et_ps[:])
                ET.append(ETt)
                yield

            # ---- phase C3: SE gate ----
            pout = []
            for cb in range(CB):
                pp = pops.tile([P, 2], f32, tag="ops", name=f"pp_{b}_{cb}")
                for db in range(CB):
                    nc.tensor.matmul(pp[:], ET[db][:, cb * P:(cb + 1) * P],
                                     px[db][:], start=(db == 0),
                                     stop=(db == CB - 1))
                pot = psm.tile([P, 2], f32r, tag="pout", name=f"pout_{b}_{cb}")
                with nc.allow_low_precision(reason="SE gate pooled term"):
                    nc.vector.tensor_scalar(out=pot[:], in0=pp[:],
                                            scalar1=rS[cb][:], scalar2=None,
                                            op0=ALU.mult)
                pout.append(pot)
                yield

            h_ps = pops.tile([CR, 2], f32, tag="ops", name=f"h_ps_{b}")
            rhs_blocks = px + pout
            for kb in range(2 * C // P):
                nc.tensor.matmul(h_ps[:], w1tr[:, kb, :], rhs_blocks[kb][:],
                                 start=(kb == 0), stop=(kb == 2 * C // P - 1))
            h_sb = psm.tile([CR, 2], f32r, tag="h", name=f"h_{b}")
            with nc.allow_low_precision(reason="SE hidden"):
                nc.scalar.activation(h_sb[:], h_ps[:], FT.Relu,
                                     bias=b1_sb[:], scale=1.0)
            yield

            se, g = [], []
            for cb in range(CB):
                z_ps = pops.tile([P, 2], f32, tag="ops", name=f"z_ps_{b}_{cb}")
                nc.tensor.matmul(z_ps[:], w2tr[:, cb * P:(cb + 1) * P], h_sb[:],
                                 start=True, stop=True)
                # sigmoid(z + b2) = 1 / (1 + exp(-z - b2)); b2n = -b2
                en = psm.tile([P, 1], f32, tag="en", name=f"en_{b}_{cb}")
                nc.scalar.activation(en[:], z_ps[:, 0:1], FT.Exp,
                                     bias=b2n_sb[:, cb:cb + 1], scale=-1.0)
                den = psm.tile([P, 1], f32, tag="den", name=f"den_{b}_{cb}")
                nc.vector.tensor_scalar_add(den[:], en[:], 1.0)
                set_ = psm.tile([P, 1], f32, tag="se", name=f"se_{b}_{cb}")
                nc.vector.reciprocal(set_[:], den[:])
                onems = psm.tile([P, 1], f32, tag="onems", name=f"om_{b}_{cb}")
                nc.vector.tensor_scalar(out=onems[:], in0=set_[:], scalar1=-1.0,
                                        scalar2=1.0, op0=ALU.mult, op1=ALU.add)
                gt = psm.tile([P, 1], f32, tag="g", name=f"g_{b}_{cb}")
                nc.vector.tensor_mul(gt[:], onems[:], rS[cb][:])
                se.append(set_)
                g.append(gt)
                yield

            # ---- phase C4: A = g*E~ + diag(se); ATg = A^T ----
            Eg, dg = [], []
            for cb in range(CB):
                Egt = pEg.tile([P, C], f32r, tag="Eg", name=f"Eg_{b}_{cb}")
                nc.vector.tensor_scalar(out=Egt[:], in0=E_sb[cb][:],
                                        scalar1=g[cb][:], scalar2=None,
                                        op0=ALU.mult)
                Eg.append(Egt)
                dgt = psm.tile([P, P], f32r, tag="dg", name=f"dg_{b}_{cb}")
                nc.vector.tensor_scalar(out=dgt[:], in0=ident[:],
                                        scalar1=se[cb][:], scalar2=None,
                                        op0=ALU.mult)
                dg.append(dgt)
            ATg = []
            for db in range(CB):
                at_ps = ptps.tile([P, C], f32r, tag="tps", name=f"at_{b}_{db}")
                for i in range(CB):
                    nc.tensor.matmul(
                        at_ps[:, i * P:(i + 1) * P],
                        Eg[i][:, db * P:(db + 1) * P], identr[:],
                        is_transpose=True, start=(i == 0), stop=(i == CB - 1))
                At = pA.tile([P, C], f32r, tag="ATg", name=f"ATg_{b}_{db}")
                if db % 2 == 0:
                    nc.scalar.activation(At[:], at_ps[:], FT.Copy)
                else:
                    nc.vector.tensor_copy(At[:], at_ps[:])
                nc.vector.tensor_add(At[:, db * P:(db + 1) * P],
                                     At[:, db * P:(db + 1) * P], dg[db][:])
                ATg.append(At)
                yield
            out.append(ATg)

        def phaseD_gen(b, ATg, deep_psum=False):
            """Generator: yields after each (nb, cb) group (32 yields)."""
            q = qr[b]
            i = 0
            for nb in range(NB):
                for cb in range(CB):
                    # the last batch's phase D has the whole PSUM to itself:
                    # alternate over the energy banks too for a deeper ring
                    tag = "ops"
                    if deep_psum and i % 2 == 1:
                        tag = "eps"
                    o_ps = pops.tile([P, 512], f32, tag=tag,
                                     name=f"o_ps_{b}_{nb}_{cb}") \
                        if tag == "ops" else \
                        peps.tile([P, 512], f32, tag=tag,
                                  name=f"o_ps_{b}_{nb}_{cb}")
                    for db in range(CB):
                        nc.tensor.matmul(
                            o_ps[:], ATg[db][:, cb * P:(cb + 1) * P],
                            q[db][:, nb * 512:(nb + 1) * 512],
                            start=(db == 0), stop=(db == CB - 1))
                    f_ = pbl.tile([P, 512], f32, tag="f",
                                  name=f"f_{b}_{nb}_{cb}")
                    if cb % 2 == 0:
                        nc.scalar.activation(f_[:], o_ps[:], FT.Copy)
                    else:
                        nc.vector.tensor_copy(f_[:], o_ps[:])
                    nc.sync.dma_start(
                        y_d[b, cb * P:(cb + 1) * P, nb * 512:(nb + 1) * 512],
                        f_[:])
                    i += 1
                    yield

        # ---- schedule ----
        # head: ident, first q chunks, then weights, then remaining chunks
        pairs0 = make_load_pairs(0)
        for i, p in enumerate(pairs0[:CB]):
            p(i)
        weights = emit_weight_loads()
        for i, p in enumerate(pairs0[CB:], start=CB):
            p(i)
        px0 = emit_px(0)
        sprinkle1 = None
        if BP > 1:
            pairs1 = make_load_pairs(1)
            sprinkle1 = {i: [pairs1[i]] for i in range(len(pairs1))}

        # B0 with b1's loads sprinkled in
        for _ in phaseB_gen(0, sprinkle=sprinkle1):
            pass

        aout0, aout1 = [], []
        if BP == 1:
            for _ in phaseC_gen(0, px0, weights, aout0):
                pass
            for _ in phaseD_gen(0, aout0[0], deep_psum=True):
                pass
        else:
            px1 = emit_px(1)
            gB1 = phaseB_gen(1)
            # C0 woven with B1 (B1's PE work fills C0's dependency stalls)
            for _ in phaseC_gen(0, px0, weights, aout0):
                next(gB1, None)
            # B1 remainder woven with D0
            gD0 = phaseD_gen(0, aout0[0])
            while next(gB1, "end") != "end":
                next(gD0, None)
            # C1 woven with D0 remainder
            for _ in phaseC_gen(1, px1, weights, aout1):
                next(gD0, None)
            for _ in gD0:
                pass
            for _ in phaseD_gen(1, aout1[0], deep_psum=True):
                pass


def _get_program():
    with _lock:
        if "nc" not in _cached:
            _cached["nc"] = _build()
    return _cached["nc"]


def _prep_in_maps(x, w1, b1, w2, b2):
    x = np.ascontiguousarray(np.asarray(x, dtype=np.float32)).reshape(B, C, N)
    w1 = np.asarray(w1, dtype=np.float32)
    b1 = np.asarray(b1, dtype=np.float32)
    w2 = np.asarray(w2, dtype=np.float32)
    b2 = np.asarray(b2, dtype=np.float32)

    w1t = (np.ascontiguousarray(w1.T) / np.float32(N)).astype(np.float32)
    w2t = np.ascontiguousarray(w2.T)
    b1c = np.ascontiguousarray(b1.reshape(CR, 1))
    b2n = np.ascontiguousarray(-b2.reshape(C, 1))
    ident = np.eye(P, dtype=np.float32)

    in_maps = []
    for c in range(NCORES):
        in_maps.append({
            "x": np.ascontiguousarray(x[c * BP:(c + 1) * BP]),
            "w1t": w1t,
            "b1": b1c,
            "w2t": w2t,
            "b2n": b2n,
            "ident": ident,
        })
    return in_maps


def run(x, w1, b1, w2, b2, trace=False):
    nc = _get_program()
    in_maps = _prep_in_maps(x, w1, b1, w2, b2)
    res = run_bass_kernel_spmd(nc, in_maps, core_ids=list(range(NCORES)),
                               trace=trace)
    y = np.concatenate([res.results[c]["y"][None] for c in range(NCORES)], axis=0)
    y = y.reshape(B, C, H, W).astype(np.float32)
    return y, res


def kernel(x, w1, b1, w2, b2):
    y, _ = run(x, w1, b1, w2, b2, trace=False)
    return y


# revision 51
# speedup vs baseline: 920197.8107x; 1.0413x over previous
"""Trainium2 Bass kernel for nn_CA_Module (DANet CAM + SE gate).

Reference math (per batch item b):
    q = x[b].reshape(C, N)                         # C=512, N=4096
    energy = q @ q.T                               # [C, C]
    att = softmax(max_row(energy) - energy)        # == softmax(-energy) rows
    out = att @ q                                  # [C, N]
    pooled = concat([mean_n x, mean_n out])        # [2C]
    hidden = relu(pooled @ w1.T + b1)              # [CR]
    se = sigmoid(hidden @ w2.T + b2)               # [C]
    y = se * x + (1 - se) * out

Sharding: data-parallel over B=16 across 8 cores (2 batch items/core).

Device implementation highlights:
  - One f32r copy of q serves everything: DMA lands raw f32 bits in the q_r
    tile, an in-place Copy rounds to f32r (satisfying the fp32r-producer
    rule) and its accum_out gives the pooled-x sums for free.
  - energy is computed upper-block-triangle only (it is symmetric); the
    missing blocks are mirrored with PE transposes into the same PSUM banks.
  - att row c: exp(min_row(energy)[c] - energy[c,:]) / S[c]; S comes free
    from the exp's accum_out.
  - final blend folds into the second matmul: y = A @ q with
    A = ((1-se)/S) * E~ + diag(se), so phase D is pure matmul + copy-out.
  - mean_n out is not reduced from the big out tensor: sum_n out_raw =
    E~ @ (sum_n q), a [512,512]@[512,2] matmul against the pooled-x sums.
  - sigmoid is exp(-z) -> +1 -> reciprocal so the ACT engine stays on the
    exp table set the whole kernel (table switches cost ~2.7us).
  - w1t is pre-scaled by 1/N on the host so pooled sums need no extra scale.
"""
import threading
import numpy as np

import concourse.bass as bass
import concourse.tile as tile
from concourse import bacc, mybir, masks
from concourse.bass_utils import run_bass_kernel_spmd

B, C, H, W = 16, 512, 64, 64
N = H * W                 # 4096
NCORES = 8
BP = B // NCORES          # batch items per core
CR = C // 8               # 64
P = 128                   # partitions
CB = C // P               # 4 c-blocks
NK = N // P               # 32 n-blocks of 128
NB = N // 512             # 8 n-chunks of 512
QCH = 8                   # q DMA/cast chunks per c-block
QCW = N // QCH            # chunk width (512)

f32 = mybir.dt.float32
f32r = mybir.dt.float32r
FT = mybir.ActivationFunctionType
ALU = mybir.AluOpType
AX = mybir.AxisListType

_lock = threading.Lock()
_cached = {}


def _build():
    nc = bacc.Bacc("TRN2", target_bir_lowering=False, debug=False,
                   num_devices=NCORES)

    x_d = nc.dram_tensor("x", [BP, C, N], f32, kind="ExternalInput").ap()
    w1t_d = nc.dram_tensor("w1t", [2 * C, CR], f32, kind="ExternalInput").ap()
    b1_d = nc.dram_tensor("b1", [CR, 1], f32, kind="ExternalInput").ap()
    w2t_d = nc.dram_tensor("w2t", [CR, C], f32, kind="ExternalInput").ap()
    b2n_d = nc.dram_tensor("b2n", [C, 1], f32, kind="ExternalInput").ap()
    ident_d = nc.dram_tensor("ident", [P, P], f32, kind="ExternalInput").ap()
    y_d = nc.dram_tensor("y", [BP, C, N], f32, kind="ExternalOutput").ap()

    with tile.TileContext(nc) as tc:
        _emit(nc, tc, x_d, w1t_d, b1_d, w2t_d, b2n_d, ident_d, y_d)
    nc.compile()
    return nc


def _emit(nc, tc, x_d, w1t_d, b1_d, w2t_d, b2n_d, ident_d, y_d):
    from contextlib import ExitStack
    ctx = ExitStack()
    with ctx:
        consts = ctx.enter_context(tc.tile_pool(name="consts", bufs=1))
        pq = ctx.enter_context(tc.tile_pool(name="pq", bufs=2 * CB))
        pst = ctx.enter_context(tc.tile_pool(name="pst", bufs=4))
        pqT = ctx.enter_context(tc.tile_pool(name="pqT", bufs=4))
        pE = ctx.enter_context(tc.tile_pool(name="pE", bufs=4))
        pET = ctx.enter_context(tc.tile_pool(name="pET", bufs=4))
        pA = ctx.enter_context(tc.tile_pool(name="pA", bufs=4))
        pEg = ctx.enter_context(tc.tile_pool(name="pEg", bufs=4))
        pmir = ctx.enter_context(tc.tile_pool(name="pmir", bufs=1))
        pbl = ctx.enter_context(tc.tile_pool(name="pbl", bufs=6))
        psm = ctx.enter_context(tc.tile_pool(name="psm", bufs=8))
        # PSUM: eps(4) + tps(2) + ops(2) = 8 banks
        peps = ctx.enter_context(
            tc.tile_pool(name="peps", bufs=4, space=bass.MemorySpace.PSUM))
        ptps = ctx.enter_context(
            tc.tile_pool(name="ptps", bufs=2, space=bass.MemorySpace.PSUM))
        pops = ctx.enter_context(
            tc.tile_pool(name="pops", bufs=2, space=bass.MemorySpace.PSUM))

        # ---- constants (weights DMA'd later, after the first q chunks) ----
        ident = consts.tile([P, P], f32, tag="ident")
        nc.sync.dma_start(ident[:], ident_d[:])
        identr = consts.tile([P, P], f32r, tag="identr")
        nc.vector.tensor_copy(identr[:], ident[:])

        def emit_weight_loads():
            w1t_sb = consts.tile([P, 2 * C // P, CR], f32, tag="w1t",
                                 name="w1t_sb")
            nc.sync.dma_start(w1t_sb[:],
                              w1t_d.rearrange("(kb p) j -> p kb j", p=P))
            w1tr = consts.tile([P, 2 * C // P, CR], f32r, tag="w1tr",
                               name="w1tr")
            nc.vector.tensor_copy(w1tr[:], w1t_sb[:])

            w2t_sb = consts.tile([CR, C], f32, tag="w2t", name="w2t_sb")
            nc.sync.dma_start(w2t_sb[:], w2t_d[:])
            w2tr = consts.tile([CR, C], f32r, tag="w2tr", name="w2tr")
            nc.vector.tensor_copy(w2tr[:], w2t_sb[:])

            b1_sb = consts.tile([CR, 1], f32, tag="b1", name="b1_sb")
            nc.sync.dma_start(b1_sb[:], b1_d[:])
            b2n_sb = consts.tile([P, CB], f32, tag="b2n", name="b2n_sb")
            nc.sync.dma_start(b2n_sb[:],
                              b2n_d.rearrange("(cb p) one -> p (cb one)", p=P))
            return w1tr, w2tr, b1_sb, b2n_sb

        # ---- per-batch state ----
        qr = {}      # b -> [CB] tiles [P, N] f32r
        pxacc = {}   # b -> [CB] accumulator tiles [P, QCH] f32

        def make_load_pairs(b):
            """Closures, one per (chunk, cb): DMA x chunk -> staging, then
            cast staging -> q_r chunk (accumulating pooled-x row sums)."""
            tiles = []
            for cb in range(CB):
                t = pq.tile([P, N], f32r, tag="q", name=f"q_{b}_{cb}")
                tiles.append(t)
            qr[b] = tiles
            pxacc[b] = [psm.tile([P, QCH], f32, tag="pxacc",
                                 name=f"pxacc_{b}_{cb}") for cb in range(CB)]

            def pair(cb, ch):
                def go(i):
                    st = pst.tile([P, QCW], f32, tag="qst",
                                  name=f"qst_{b}_{cb}_{ch}")
                    nc.sync.dma_start(
                        st[:],
                        x_d[b, cb * P:(cb + 1) * P, ch * QCW:(ch + 1) * QCW])
                    dst = tiles[cb][:, ch * QCW:(ch + 1) * QCW]
                    acc = pxacc[b][cb][:, ch:ch + 1]
                    with nc.allow_low_precision(reason="f32r round of q"):
                        nc.vector.tensor_scalar(
                            out=dst, in0=st[:], scalar1=1.0, scalar2=0.0,
                            op0=ALU.mult, op1=ALU.add, accum_out=acc)
                return go

            return [pair(cb, ch) for ch in range(QCH) for cb in range(CB)]

        def emit_px(b):
            px = []
            for cb in range(CB):
                pxt = psm.tile([P, 2], f32r, tag="px", name=f"px_{b}_{cb}")
                with nc.allow_low_precision(reason="pooled sums feed SE gate"):
                    nc.vector.tensor_reduce(pxt[:, 0:1], pxacc[b][cb][:],
                                            axis=AX.X, op=ALU.add)
                    nc.vector.tensor_copy(pxt[:, 1:2], pxt[:, 0:1])
                px.append(pxt)
            return px

        E_ps_of = {}

        def phaseB_gen(b, sprinkle=None):
            """Generator: yields after each k iteration (32 yields)."""
            q = qr[b]
            E_ps = [peps.tile([P, C], f32, tag="eps", name=f"E_ps_{b}_{i}")
                    for i in range(CB)]
            E_ps_of[b] = E_ps

            def emit_transpose_block(k):
                t_ps = ptps.tile([P, C], f32r, tag="tps", name=f"t_ps_{b}_{k}")
                for cb in range(CB):
                    nc.tensor.transpose(
                        t_ps[:, cb * P:(cb + 1) * P],
                        q[cb][:, k * P:(k + 1) * P], identr[:])
                qTt = pqT.tile([P, C], f32r, tag="qT", name=f"qT_{b}_{k}")
                nc.scalar.activation(qTt[:], t_ps[:], FT.Copy)
                return qTt

            prev_qT = emit_transpose_block(0)
            for k in range(NK):
                nxt_qT = emit_transpose_block(k + 1) if k + 1 < NK else None
                for mc in range(CB):
                    nc.tensor.matmul(
                        E_ps[mc][:, mc * P:C],
                        prev_qT[:, mc * P:(mc + 1) * P],
                        prev_qT[:, mc * P:C],
                        start=(k == 0), stop=(k == NK - 1))
                if sprinkle is not None:
                    for j, fn in enumerate(sprinkle.get(k, ())):
                        fn(k + j)
                prev_qT = nxt_qT
                yield

        def phaseC_gen(b, px, weights, out):
            w1tr, w2tr, b1_sb, b2n_sb = weights
            E_ps = E_ps_of[b]
            # mirror lower-triangle blocks: E[md, mc] = E[mc, md]^T
            for mc in range(CB):
                for md in range(mc + 1, CB):
                    mt = pmir.tile([P, P], f32, tag="mir",
                                   name=f"mir_{b}_{mc}_{md}")
                    nc.vector.tensor_copy(mt[:], E_ps[mc][:, md * P:(md + 1) * P])
                    nc.tensor.matmul(
                        E_ps[md][:, mc * P:(mc + 1) * P], mt[:], ident[:],
                        is_transpose=True, start=False, stop=True)
                yield
            # ---- phase C: softmax pieces ----
            E_sb, rS = [], []
            for mc in range(CB):
                m_sb = psm.tile([P, 1], f32, tag="m", name=f"m_{b}_{mc}")
                nc.vector.tensor_reduce(m_sb[:], E_ps[mc][:], axis=AX.X,
                                        op=ALU.min)
                Et = pE.tile([P, C], f32r, tag="E", name=f"E_{b}_{mc}")
                S_sb = psm.tile([P, 1], f32, tag="S", name=f"S_{b}_{mc}")
                nc.scalar.activation(Et[:], E_ps[mc][:], FT.Exp,
                                     bias=m_sb[:], scale=-1.0, accum_out=S_sb[:])
                rSt = psm.tile([P, 1], f32, tag="rS", name=f"rS_{b}_{mc}")
                nc.vector.reciprocal(rSt[:], S_sb[:])
                E_sb.append(Et)
                rS.append(rSt)
                yield

            # ---- phase C2: ET = E~^T (for the SE pooled-out term) ----
            ET = []
            for db in range(CB):
                et_ps = ptps.tile([P, C], f32r, tag="tps", name=f"et_{b}_{db}")
                for cb in range(CB):
                    nc.tensor.transpose(
                        et_ps[:, cb * P:(cb + 1) * P],
                        E_sb[cb][:, db * P:(db + 1) * P], identr[:])
                ETt = pET.tile([P, C], f32r, tag="ET", name=f"ET_{b}_{db}")
                if db % 2 == 0:
                    nc.scalar.activation(ETt[:], et_ps[:], FT.Copy)
                else:
                    nc.vector.tensor_copy(ETt[:], et_ps[:])
                ET.append(ETt)
                yield

            # ---- phase C3: SE gate ----
            pout = []
            for cb in range(CB):
                pp = pops.tile([P, 2], f32, tag="ops", name=f"pp_{b}_{cb}")
                for db in range(CB):
                    nc.tensor.matmul(pp[:], ET[db][:, cb * P:(cb + 1) * P],
                                     px[db][:], start=(db == 0),
                                     stop=(db == CB - 1))
                pot = psm.tile([P, 2], f32r, tag="pout", name=f"pout_{b}_{cb}")
                with nc.allow_low_precision(reason="SE gate pooled term"):
                    nc.vector.tensor_scalar(out=pot[:], in0=pp[:],
                                            scalar1=rS[cb][:], scalar2=None,
                                            op0=ALU.mult)
                pout.append(pot)
                yield

            h_ps = pops.tile([CR, 2], f32, tag="ops", name=f"h_ps_{b}")
            rhs_blocks = px + pout
            for kb in range(2 * C // P):
                nc.tensor.matmul(h_ps[:], w1tr[:, kb, :], rhs_blocks[kb][:],
                                 start=(kb == 0), stop=(kb == 2 * C // P - 1))
            h_sb = psm.tile([CR, 2], f32r, tag="h", name=f"h_{b}")
            with nc.allow_low_precision(reason="SE hidden"):
                nc.scalar.activation(h_sb[:], h_ps[:], FT.Relu,
                                     bias=b1_sb[:], scale=1.0)
            yield

            se, g = [], []
            for cb in range(CB):
                z_ps = pops.tile([P, 2], f32, tag="ops", name=f"z_ps_{b}_{cb}")
                nc.tensor.matmul(z_ps[:], w2tr[:, cb * P:(cb + 1) * P], h_sb[:],
                                 start=True, stop=True)
                # sigmoid(z + b2) = 1 / (1 + exp(-z - b2)); b2n = -b2
                en = psm.tile([P, 1], f32, tag="en", name=f"en_{b}_{cb}")
                nc.scalar.activation(en[:], z_ps[:, 0:1], FT.Exp,
                                     bias=b2n_sb[:, cb:cb + 1], scale=-1.0)
                den = psm.tile([P, 1], f32, tag="den", name=f"den_{b}_{cb}")
                nc.vector.tensor_scalar_add(den[:], en[:], 1.0)
                set_ = psm.tile([P, 1], f32, tag="se", name=f"se_{b}_{cb}")
                nc.vector.reciprocal(set_[:], den[:])
                onems = psm.tile([P, 1], f32, tag="onems", name=f"om_{b}_{cb}")
                nc.vector.tensor_scalar(out=onems[:], in0=set_[:], scalar1=-1.0,
                                        scalar2=1.0, op0=ALU.mult, op1=ALU.add)
                gt = psm.tile([P, 1], f32, tag="g", name=f"g_{b}_{cb}")
                nc.vector.tensor_mul(gt[:], onems[:], rS[cb][:])
                se.append(set_)
                g.append(gt)
                yield

            # ---- phase C4: A = g*E~ + diag(se); ATg = A^T ----
            Eg, dg = [], []
            for cb in range(CB):
                Egt = pEg.tile([P, C], f32r, tag="Eg", name=f"Eg_{b}_{cb}")
                nc.vector.tensor_scalar(out=Egt[:], in0=E_sb[cb][:],
                                        scalar1=g[cb][:], scalar2=None,
                                        op0=ALU.mult)
                Eg.append(Egt)
                dgt = psm.tile([P, P], f32r, tag="dg", name=f"dg_{b}_{cb}")
                nc.vector.tensor_scalar(out=dgt[:], in0=ident[:],
                                        scalar1=se[cb][:], scalar2=None,
                                        op0=ALU.mult)
                dg.append(dgt)
            ATg = []
            for db in range(CB):
                at_ps = ptps.tile([P, C], f32r, tag="tps", name=f"at_{b}_{db}")
                for i in range(CB):
                    nc.tensor.matmul(
                        at_ps[:, i * P:(i + 1) * P],
                        Eg[i][:, db * P:(db + 1) * P], identr[:],
                        is_transpose=True, start=(i == 0), stop=(i == CB - 1))
                At = pA.tile([P, C], f32r, tag="ATg", name=f"ATg_{b}_{db}")
                if db % 2 == 0:
                    nc.scalar.activation(At[:], at_ps[:], FT.Copy)
                else:
                    nc.vector.tensor_copy(At[:], at_ps[:])
                nc.vector.tensor_add(At[:, db * P:(db + 1) * P],
                                     At[:, db * P:(db + 1) * P], dg[db][:])
                ATg.append(At)
                yield
            out.append(ATg)

        def phaseD_gen(b, ATg, deep_psum=False):
            """Generator: yields after each (nb, cb) group (32 yields)."""
            q = qr[b]
            i = 0
            for nb in range(NB):
                for cb in range(CB):
                    # the last batch's phase D has the whole PSUM to itself:
                    # alternate over the energy banks too for a deeper ring
                    tag = "ops"
                    if deep_psum and i % 2 == 1:
                        tag = "eps"
                    o_ps = pops.tile([P, 512], f32, tag=tag,
                                     name=f"o_ps_{b}_{nb}_{cb}") \
                        if tag == "ops" else \
                        peps.tile([P, 512], f32, tag=tag,
                                  name=f"o_ps_{b}_{nb}_{cb}")
                    for db in range(CB):
                        nc.tensor.matmul(
                            o_ps[:], ATg[db][:, cb * P:(cb + 1) * P],
                            q[db][:, nb * 512:(nb + 1) * 512],
                            start=(db == 0), stop=(db == CB - 1))
                    f_ = pbl.tile([P, 512], f32, tag="f",
                                  name=f"f_{b}_{nb}_{cb}")
                    if cb % 2 == 0:
                        nc.scalar.activation(f_[:], o_ps[:], FT.Copy)
                    else:
                        nc.vector.tensor_copy(f_[:], o_ps[:])
                    nc.sync.dma_start(
                        y_d[b, cb * P:(cb + 1) * P, nb * 512:(nb + 1) * 512],
                        f_[:])
                    i += 1
                    yield

        # ---- schedule ----
        # head: ident, first q chunks, then weights, then remaining chunks
        pairs0 = make_load_pairs(0)
        for i, p in enumerate(pairs0[:CB]):
            p(i)
        weights = emit_weight_loads()
        for i, p in enumerate(pairs0[CB:], start=CB):
            p(i)
        px0 = emit_px(0)
        sprinkle1 = None
        if BP > 1:
            pairs1 = make_load_pairs(1)
            sprinkle1 = {i: [pairs1[i]] for i in range(len(pairs1))}

        # B0 with b1's loads sprinkled in
        for _ in phaseB_gen(0, sprinkle=sprinkle1):
            pass

        aout0, aout1 = [], []
        if BP == 1:
            for _ in phaseC_gen(0, px0, weights, aout0):
                pass
            for _ in phaseD_gen(0, aout0[0], deep_psum=True):
                pass
        else:
            px1 = emit_px(1)
            gB1 = phaseB_gen(1)
            # C0 woven with B1 (B1's PE work fills C0's dependency stalls)
            for _ in phaseC_gen(0, px0, weights, aout0):
                next(gB1, None)
            # B1 remainder woven with D0
            gD0 = phaseD_gen(0, aout0[0])
            while next(gB1, "end") != "end":
                next(gD0, None)
            # C1 woven with D0 remainder
            for _ in phaseC_gen(1, px1, weights, aout1):
                next(gD0, None)
            for _ in gD0:
                pass
            for _ in phaseD_gen(1, aout1[0], deep_psum=True):
                pass


def _get_program():
    with _lock:
        if "nc" not in _cached:
            _cached["nc"] = _build()
    return _cached["nc"]


def _prep_in_maps(x, w1, b1, w2, b2):
    x = np.ascontiguousarray(np.asarray(x, dtype=np.float32)).reshape(B, C, N)
    w1 = np.asarray(w1, dtype=np.float32)
    b1 = np.asarray(b1, dtype=np.float32)
    w2 = np.asarray(w2, dtype=np.float32)
    b2 = np.asarray(b2, dtype=np.float32)

    w1t = (np.ascontiguousarray(w1.T) / np.float32(N)).astype(np.float32)
    w2t = np.ascontiguousarray(w2.T)
    b1c = np.ascontiguousarray(b1.reshape(CR, 1))
    b2n = np.ascontiguousarray(-b2.reshape(C, 1))
    ident = np.eye(P, dtype=np.float32)

    in_maps = []
    for c in range(NCORES):
        in_maps.append({
            "x": np.ascontiguousarray(x[c * BP:(c + 1) * BP]),
            "w1t": w1t,
            "b1": b1c,
            "w2t": w2t,
            "b2n": b2n,
            "ident": ident,
        })
    return in_maps


def run(x, w1, b1, w2, b2, trace=False):
    nc = _get_program()
    in_maps = _prep_in_maps(x, w1, b1, w2, b2)
    res = run_bass_kernel_spmd(nc, in_maps, core_ids=list(range(NCORES)),
                               trace=trace)
    y = np.concatenate([res.results[c]["y"][None] for c in range(NCORES)], axis=0)
    y = y.reshape(B, C, H, W).astype(np.float32)
    return y, res


def kernel(x, w1, b1, w2, b2):
    y, _ = run(x, w1, b1, w2, b2, trace=False)
    return y
